# revision 11
# baseline (speedup 1.0000x reference)
"""BEiT attention block on 8 TRN2 NeuronCores, data-parallel over batch.

Full inputs -> kernel(**inputs) -> full output (16, 1025, 768) f32.

Per-core work: 2 batches of multi-head attention (N=1025 tokens, C=768,
H=12 heads, d=64) with a relative-position bias added to the logits.

v2 structure (qc-major pipeline):
  - all matmul operands bf16, f32 PSUM accumulation; softmax folded into
    exp(s*scale) * exp(bias) with exp(bias^T) precomputed in bf16.
  - k-projections persist across query chunks; q projected per chunk.
  - query-chunk-major loop: normalize + output projection for chunk qc
    overlap the attention compute of chunk qc+1, shrinking the tail.
  - v projected per 6-head group, interleaved into the first chunk's
    attention so PV never waits on the full v projection.
  - softmax denominators via the ones-column of v; reciprocal rows are
    computed straight out of PSUM (no staging copy), DMA'd to DRAM, and
    broadcast back with a partition-step-0 access pattern.
  - big EB-table loads ride the SP DMA queue; small/latency-critical
    transfers ride the Activation DMA queue.
"""

import numpy as np
import ml_dtypes

B = 16
N = 1025
C = 768
H = 12
D = 64
NCORES = 8
BPC = B // NCORES  # batches per core
NPAD = 1152        # padded key length: 9 * 128
KB = NPAD // 128   # key blocks
IB = C // 128      # input-channel blocks
NQ = 1026          # query extent incl. one even-ing pad column
QCS = [384, 384, 258]   # query chunks covering NQ (all even -> DVE 2x mode)
QCO = [0, 384, 768]
KCS = [342, 342, 342]   # k-projection chunks covering NQ
KCO = [0, 342, 684]
SCALE = D ** -0.5
# minimax cubic p(z) ~ exp(z) on [-0.55, 0.55], factored a3*(z-R)*(z^2+S*z+T);
# exp(y)*EB computed as (p(y/4) * a3*EB^(1/4))^4 in one custom DVE op.
EXP_R = -1.658048394110858
EXP_S = 1.462861309003841
EXP_T = 3.672443055287797
EXP_A3 = 0.1641584267735188
BF16 = ml_dtypes.bfloat16

_cache: dict = {}


def _register_exp4():
    """Register the fused quartic-exp custom DVE op: out = (p(z)*Src1)^4
    with p monic-factored; Src1 carries a3*EB^(1/4)."""
    if "exp4" in _cache:
        return _cache["exp4"]
    import numpy as np
    from concourse import dve_ops
    from concourse.dve_spec import Spec, Src0, Src1, C0, C1, C2
    from concourse.dve_table_gen import dve_ver_for
    from concourse.dve_uop import DveOpSpec

    name = "EXP4_EB_ANT"
    for op in dve_ops.OPS:
        if op.name == name:
            _cache["exp4"] = op
            return op

    m5 = (((Src0 + C1) * Src0) + C2) * (Src0 - C0) * Src1
    p2 = m5 * m5
    body = p2 * p2

    def ref(in0, in1, s0, s1, imm2):
        m5 = ((((in0 + s1) * in0) + imm2) * (in0 - s0) * in1).astype(np.float32)
        p2 = (m5 * m5).astype(np.float32)
        return (p2 * p2).astype(np.float32)

    op = dve_ops.DveOp(name, Spec(body=body, reference=ref), subdim=False,
                       uops_sha={})
    row = dve_ops._CUSTOM_DVE_ROW_BASE + len(dve_ops.OPS)
    assert row < 0x20
    dve_ops.OPS.append(op)
    dve_ops._SUB_OPCODE_FOR_NAME[name] = row
    dve_ops.CUSTOM_DVE_SPECS[name] = op.spec
    ver = dve_ver_for("TRN2")
    spec = DveOpSpec(name=name, opcode=row,
                     uops=dve_ops.lower(op.spec, ver=ver),
                     rd1_en=dve_ops.has_src1(op.spec))
    op.uops_sha[ver] = spec.sha(ver)
    _cache["exp4"] = op
    return op


def _build():
    import concourse.bass as bass
    import concourse.mybir as mybir
    import concourse.tile as tile
    from concourse import bacc

    dt = mybir.dt
    f32 = dt.float32
    bf = dt.bfloat16
    AFT = mybir.ActivationFunctionType

    exp4_op = _register_exp4()
    nc = bacc.Bacc("TRN2", target_bir_lowering=False, debug=False)

    xT_d = nc.declare_dram_parameter("xT", [BPC, C, NPAD], bf, isOutput=False)
    wqk_d = nc.declare_dram_parameter("wqk", [C, 2 * C], bf, isOutput=False)
    wv_d = nc.declare_dram_parameter("wv", [C, C], bf, isOutput=False)
    wp_d = nc.declare_dram_parameter("wp", [C, C], bf, isOutput=False)
    qkb_d = nc.declare_dram_parameter("qkb", [128, 2 * IB], f32, isOutput=False)
    vb_d = nc.declare_dram_parameter("vb", [128, C], f32, isOutput=False)
    pb_d = nc.declare_dram_parameter("pb", [128, C], f32, isOutput=False)
    eb_d = nc.declare_dram_parameter("eb", [H, NPAD, NQ], bf, isOutput=False)
    out_d = nc.declare_dram_parameter("out", [BPC, N, C], f32, isOutput=True)

    with tile.TileContext(nc) as tc:
        from contextlib import ExitStack

        ctx = ExitStack()
        with ctx:
            consts = ctx.enter_context(tc.tile_pool(name="consts", bufs=1))
            persist = ctx.enter_context(tc.tile_pool(name="persist", bufs=1))

            # ---- constants ----
            wqk_sb = consts.tile([128, IB, 2 * C], bf)
            nc.sync.dma_start(wqk_sb[:], wqk_d.ap().rearrange("(ib p) o -> p ib o", p=128))
            qkb_sb = consts.tile([128, 2 * IB], f32)
            nc.scalar.dma_start(qkb_sb[:], qkb_d.ap())
            vb_sb = consts.tile([128, C], f32)
            nc.scalar.dma_start(vb_sb[:], vb_d.ap())
            pb_sb = consts.tile([128, C], f32)
            nc.scalar.dma_start(pb_sb[:], pb_d.ap())
            wv_sb = consts.tile([128, IB, C], bf)
            nc.sync.dma_start(wv_sb[:], wv_d.ap().rearrange("(ib p) o -> p ib o", p=128))
            wp_sb = consts.tile([128, IB, C], bf)
            nc.sync.dma_start(wp_sb[:], wp_d.ap().rearrange("(ib p) o -> p ib o", p=128))

            # ---- persistent tensors ----
            xT_sb = []
            v_sb = []   # v_sb[b][vc]: [128, KB, 6, D+1]
            k_sb = []   # k_sb[hp][b]: [128, NPAD]
            for b in range(BPC):
                t = persist.tile([128, IB, NPAD], bf, name=f"xT{b}")
                for ib in range(IB):
                    nc.sync.dma_start(
                        t[:, ib, :], xT_d.ap()[b][ib * 128:(ib + 1) * 128, :])
                xT_sb.append(t)
                vt = [persist.tile([128, KB, 6, D + 1], bf, name=f"v{b}_{vc}")
                      for vc in range(2)]
                for vc in range(2):
                    nc.gpsimd.memset(vt[vc][:, :, :, D:], 1.0)
                v_sb.append(vt)
            for hp in range(H // 2):
                k_sb.append([persist.tile([128, NPAD], bf, name=f"k{hp}_{b}")
                             for b in range(BPC)])
                for b in range(BPC):
                    nc.gpsimd.memset(k_sb[hp][b][:, NQ:], 0.0)

            # recip_dram[b*H + h, q] = bf16 reciprocal softmax denominators
            recip_dram = nc.dram_tensor("recip_dram", [BPC * H, NQ], bf)

            # a tiles per (b, qc): [128, 6, qcs]; y/proj resources
            a_pool = ctx.enter_context(tc.tile_pool(name="a", bufs=4))
            rb_pool = ctx.enter_context(tc.tile_pool(name="rb", bufs=1))
            y_pool = ctx.enter_context(tc.tile_pool(name="y", bufs=2))
            sums_pool = ctx.enter_context(tc.tile_pool(name="sums", bufs=2))
            stg_pool = ctx.enter_context(tc.tile_pool(name="stg", bufs=2))
            q_pool = ctx.enter_context(tc.tile_pool(name="q", bufs=3))
            eb_pool = ctx.enter_context(tc.tile_pool(name="ebp", bufs=3))
            ex_pool = ctx.enter_context(tc.tile_pool(name="exp", bufs=3))
            et_pool = ctx.enter_context(tc.tile_pool(name="etmp", bufs=2))
            mm_ps = ctx.enter_context(tc.tile_pool(name="mm_ps", bufs=2, space="PSUM"))
            s_ps = ctx.enter_context(tc.tile_pool(name="s_ps", bufs=4, space="PSUM"))
            o_ps = ctx.enter_context(tc.tile_pool(name="o_ps", bufs=2, space="PSUM"))

            a_sb = [[None] * 3 for _ in range(BPC)]

            def kproj(hp, b):
                """k rows for head pair hp, batch b -> k_sb[hp][b][:, :NQ]."""
                for kc in range(3):
                    ps = mm_ps.tile([128, 384], f32, name="mm")
                    for ib in range(IB):
                        nc.tensor.matmul(
                            ps[:, :KCS[kc]],
                            lhsT=wqk_sb[:, ib, C + hp * 128:C + (hp + 1) * 128],
                            rhs=xT_sb[b][:, ib, KCO[kc]:KCO[kc] + KCS[kc]],
                            start=(ib == 0),
                            stop=(ib == IB - 1),
                        )
                    nc.scalar.activation(
                        out=k_sb[hp][b][:, KCO[kc]:KCO[kc] + KCS[kc]],
                        in_=ps[:, :KCS[kc]],
                        func=AFT.Identity,
                        bias=qkb_sb[:, IB + hp:IB + hp + 1],
                        scale=1.0,
                    )

            def qproj(hp, qc, b, qt):
                """q chunk qc for head pair hp, batch b -> qt [128, qcs]."""
                qcs, qco = QCS[qc], QCO[qc]
                ps = mm_ps.tile([128, 384], f32, name="mm")
                for ib in range(IB):
                    nc.tensor.matmul(
                        ps[:, :qcs],
                        lhsT=wqk_sb[:, ib, hp * 128:(hp + 1) * 128],
                        rhs=xT_sb[b][:, ib, qco:qco + qcs],
                        start=(ib == 0),
                        stop=(ib == IB - 1),
                    )
                nc.scalar.activation(
                    out=qt[:, :qcs],
                    in_=ps[:, :qcs],
                    func=AFT.Identity,
                    bias=qkb_sb[:, hp:hp + 1],
                    scale=SCALE / 4,
                )

            def vproj(b, vc):
                """v for heads 6vc..6vc+5, batch b -> v_sb[b][vc]."""
                for kpb in range(KB):
                    ps = mm_ps.tile([128, 384], f32, name="mm")
                    for ib in range(IB):
                        nc.tensor.matmul(
                            ps[:],
                            lhsT=xT_sb[b][:, ib, kpb * 128:(kpb + 1) * 128],
                            rhs=wv_sb[:, ib, vc * 384:(vc + 1) * 384],
                            start=(ib == 0),
                            stop=(ib == IB - 1),
                        )
                    nc.vector.tensor_add(
                        out=v_sb[b][vc][:, kpb, :, :D],
                        in0=ps.rearrange("p (h d) -> p h d", d=D),
                        in1=vb_sb[:, vc * 384:(vc + 1) * 384].rearrange(
                            "p (h d) -> p h d", d=D),
                    )

            def attn_segment(hp, qc, b, qt, ebt, sums, mid=None):
                """scores+exp+PV for (hp, qc, b); writes a_sb and sums rows.
                `mid` emits extra tensor work between scores and PV so it
                overlaps the exp/mul phase."""
                qcs, qco = QCS[qc], QCO[qc]
                ex = [ex_pool.tile([128, KB, 384], bf, name="ex")
                      for _ in range(2)]
                et = None
                for kb in range(KB):
                    for par in range(2):
                        p0 = par * 64
                        st = s_ps.tile([128, 384], f32, name="st")
                        nc.tensor.matmul(
                            st[:, :qcs],
                            lhsT=k_sb[hp][b][p0:p0 + 64, kb * 128:(kb + 1) * 128],
                            rhs=qt[p0:p0 + 64, :qcs],
                        )
                        if par == 0:
                            nc.vector._custom_dve(
                                exp4_op,
                                out=ex[par][:, kb, :qcs],
                                in0=st[:, :qcs],
                                in1=ebt[par][:, kb, :qcs],
                                s0=EXP_R, s1=EXP_S, imm2=EXP_T,
                            )
                        else:
                            if kb % 3 == 0:
                                et = et_pool.tile([128, 3, 384], bf, name="et")
                            nc.scalar.activation(
                                out=et[:, kb % 3, :qcs], in_=st[:, :qcs],
                                func=AFT.Exp, scale=4.0,
                            )
                            if kb % 3 == 2:
                                k0 = kb - 2
                                eng = nc.gpsimd if kb < 6 else nc.vector
                                eng.tensor_mul(
                                    out=ex[par][:, k0:k0 + 3, :qcs],
                                    in0=et[:, :, :qcs],
                                    in1=ebt[par][:, k0:k0 + 3, :qcs],
                                )
                if mid is not None:
                    mid()
                for par in range(2):
                    h = 2 * hp + par
                    po = o_ps.tile([D + 1, 384], f32, name="po")
                    for kb in range(KB):
                        nc.tensor.matmul(
                            po[:, :qcs],
                            lhsT=v_sb[b][hp // 3][:, kb, (h - 6 * (hp // 3)), :],
                            rhs=ex[par][:, kb, :qcs],
                            start=(kb == 0),
                            stop=(kb == KB - 1),
                        )
                    stg = stg_pool.tile([65, 384], f32, name="stg")
                    nc.vector.tensor_copy(
                        out=stg[64:65, :qcs], in_=po[D:D + 1, :qcs])
                    nc.scalar.dma_start(
                        sums[h:h + 1, :qcs], stg[64:65, :qcs])
                    nc.scalar.activation(
                        out=a_sb[b][qc][par * 64:(par + 1) * 64, hp, :qcs],
                        in_=po[:D, :qcs],
                        func=AFT.Copy,
                    )

            def finalize_sums(b, qc, sums):
                """reciprocal of denominators -> bf16 -> recip_dram rows."""
                qcs, qco = QCS[qc], QCO[qc]
                nc.vector.reciprocal_approx_fast(
                    out=sums[:, :qcs], in_=sums[:, :qcs])
                rqb = sums_pool.tile([H, 384], bf, name="rqb")
                nc.vector.tensor_copy(out=rqb[:, :qcs], in_=sums[:, :qcs])
                nc.scalar.dma_start(
                    recip_dram.ap()[b * H:(b + 1) * H, qco:qco + qcs],
                    rqb[:, :qcs],
                )

            def norm_proj(b, qc):
                """normalize a_sb[b][qc] and project + emit output rows."""
                qcs, qco = QCS[qc], QCO[qc]
                rb = rb_pool.tile([128, 6, 384], bf, name="rb")
                base = recip_dram.ap()
                for par in range(2):
                    bcast = bass.AP(
                        tensor=base.tensor,
                        offset=(b * H + par) * NQ + qco,
                        ap=[[0, 64], [2 * NQ, IB], [1, qcs]],
                    )
                    nc.scalar.dma_start(
                        rb[par * 64:(par + 1) * 64, :, :qcs], bcast)
                nc.vector.tensor_mul(
                    out=a_sb[b][qc][:, :, :qcs],
                    in0=a_sb[b][qc][:, :, :qcs],
                    in1=rb[:, :, :qcs],
                )
                for qt3 in range(3):
                    off = qt3 * 128
                    cols = min(128, qcs - off)
                    rows = min(cols, N - (qco + off))
                    if rows <= 0:
                        continue
                    yt = y_pool.tile([128, C], f32, name="yt")
                    for oc2 in range(2):
                        ps = mm_ps.tile([128, 384], f32, name="mm")
                        for ib in range(IB):
                            nc.tensor.matmul(
                                ps[:cols, :],
                                lhsT=a_sb[b][qc][:, ib, off:off + cols],
                                rhs=wp_sb[:, ib, oc2 * 384:(oc2 + 1) * 384],
                                start=(ib == 0),
                                stop=(ib == IB - 1),
                            )
                        nc.vector.tensor_add(
                            out=yt[:cols, oc2 * 384:(oc2 + 1) * 384],
                            in0=ps[:cols, :],
                            in1=pb_sb[:cols, oc2 * 384:(oc2 + 1) * 384],
                        )
                    nc.scalar.dma_start(
                        out_d.ap()[b][qco + off:qco + off + rows, :],
                        yt[:rows, :],
                    )

            # ---- qc-major pipeline ----
            sums_t = [[None] * 3 for _ in range(BPC)]
            for qc in range(3):
                qcs, qco = QCS[qc], QCO[qc]
                for b in range(BPC):
                    sums_t[b][qc] = sums_pool.tile([H, 384], f32,
                                                   name="sums")
                    a_sb[b][qc] = a_pool.tile(
                        [128, 6, 384], bf, name="a")
                for hp in range(H // 2):
                    # eb tiles for this (hp, qc)
                    ebt = []
                    for par in range(2):
                        h = 2 * hp + par
                        t = eb_pool.tile([128, KB, 384], bf, name="ebt")
                        nc.sync.dma_start(
                            t[:, :, :qcs],
                            eb_d.ap()[h].rearrange(
                                "(kb p) q -> p kb q", p=128)[:, :, qco:qco + qcs],
                        )
                        ebt.append(t)
                    qts = []
                    for b in range(BPC):
                        if qc == 0:
                            kproj(hp, b)
                        qt = q_pool.tile([128, 384], bf, name="qt")
                        qproj(hp, qc, b, qt)
                        qts.append(qt)
                    for b in range(BPC):
                        # v projection rides between scores and PV so it
                        # overlaps the exp/mul phase of the same segment
                        mid = None
                        if qc == 0 and hp == 0:
                            mid = (lambda bb=b: vproj(bb, 0))
                        elif qc == 0 and hp == 1:
                            mid = (lambda bb=b: vproj(bb, 1))
                        attn_segment(hp, qc, b, qts[b], ebt,
                                     sums_t[b][qc], mid=mid)
                    if hp == 5:
                        for b in range(BPC):
                            finalize_sums(b, qc, sums_t[b][qc])
                    if qc > 0 and hp == 1:
                        norm_proj(0, qc - 1)
                    elif qc > 0 and hp == 3:
                        norm_proj(1, qc - 1)
            norm_proj(0, 2)
            norm_proj(1, 2)

    nc.compile()
    return nc


def _prepare_inputs(x, qkv_weight, q_bias, v_bias, rel_pos_table, proj_weight,
                    proj_bias, rel_pos_index):
    x = np.asarray(x, np.float32)
    qkv_weight = np.asarray(qkv_weight, np.float32)
    q_bias = np.asarray(q_bias, np.float32)
    v_bias = np.asarray(v_bias, np.float32)
    rel_pos_table = np.asarray(rel_pos_table, np.float32)
    proj_weight = np.asarray(proj_weight, np.float32)
    proj_bias = np.asarray(proj_bias, np.float32)
    rel_pos_index = np.asarray(rel_pos_index)

    wqk = np.ascontiguousarray(qkv_weight[:2 * C].T).astype(BF16)
    wv = np.ascontiguousarray(qkv_weight[2 * C:].T).astype(BF16)
    wp = np.ascontiguousarray(proj_weight.T).astype(BF16)

    qkb = np.concatenate([q_bias * np.float32(SCALE / 4), np.zeros(C, np.float32)])
    qkb = np.ascontiguousarray(qkb.reshape(2 * IB, 128).T)  # [128, 12]
    vb = np.ascontiguousarray(np.broadcast_to(v_bias, (128, C)))
    pb = np.ascontiguousarray(np.broadcast_to(proj_bias, (128, C)))

    # exp of transposed rel-pos bias, padded key rows = 0
    bias_qkh = rel_pos_table[rel_pos_index.reshape(-1)].reshape(N, N, H)
    bT = np.zeros((H, N, NQ), np.float64)
    bT[:, :, :N] = bias_qkh.transpose(2, 1, 0)
    ebt = np.zeros((H, NPAD, NQ), BF16)
    for h in range(H):
        if h % 2 == 0:
            ebt[h, :N, :] = (EXP_A3 * np.exp(bT[h] / 4)).astype(BF16)
        else:
            ebt[h, :N, :] = np.exp(bT[h]).astype(BF16)

    in_maps = []
    for core in range(NCORES):
        xb = x[core * BPC:(core + 1) * BPC]
        xT = np.zeros((BPC, C, NPAD), BF16)
        xT[:, :, :N] = xb.transpose(0, 2, 1).astype(BF16)
        in_maps.append({
            "xT": xT, "wqk": wqk, "wv": wv, "wp": wp,
            "qkb": qkb, "vb": vb, "pb": pb, "eb": ebt,
        })
    return in_maps


def kernel(**inputs) -> np.ndarray:
    from concourse.bass_utils import run_bass_kernel_spmd

    if "nc" not in _cache:
        _cache["nc"] = _build()
    nc = _cache["nc"]

    in_maps = _prepare_inputs(**inputs)
    trace = bool(_cache.get("trace", False))
    res = run_bass_kernel_spmd(nc, in_maps, core_ids=list(range(NCORES)),
                               trace=trace)
    _cache["last_results"] = res
    out = np.concatenate([r["out"] for r in res.results], axis=0)
    return out.astype(np.float32)


# revision 13
# speedup vs baseline: 1.0184x; 1.0184x over previous
"""BEiT attention block on 8 TRN2 NeuronCores, data-parallel over batch.

Full inputs -> kernel(**inputs) -> full output (16, 1025, 768) f32.

Per-core work: 2 batches of multi-head attention (N=1025 tokens, C=768,
H=12 heads, d=64) with a relative-position bias added to the logits.

v2 structure (qc-major pipeline):
  - all matmul operands bf16, f32 PSUM accumulation; softmax folded into
    exp(s*scale) * exp(bias) with exp(bias^T) precomputed in bf16.
  - k-projections persist across query chunks; q projected per chunk.
  - query-chunk-major loop: normalize + output projection for chunk qc
    overlap the attention compute of chunk qc+1, shrinking the tail.
  - v projected per 6-head group, interleaved into the first chunk's
    attention so PV never waits on the full v projection.
  - softmax denominators via the ones-column of v; reciprocal rows are
    computed straight out of PSUM (no staging copy), DMA'd to DRAM, and
    broadcast back with a partition-step-0 access pattern.
  - big EB-table loads ride the SP DMA queue; small/latency-critical
    transfers ride the Activation DMA queue.
"""

import numpy as np
import ml_dtypes

B = 16
N = 1025
C = 768
H = 12
D = 64
NCORES = 8
BPC = B // NCORES  # batches per core
NPAD = 1152        # padded key length: 9 * 128
KB = NPAD // 128   # key blocks
IB = C // 128      # input-channel blocks
NQ = 1026          # query extent incl. one even-ing pad column
QCS = [384, 384, 258]   # query chunks covering NQ (all even -> DVE 2x mode)
QCO = [0, 384, 768]
KCS = [342, 342, 342]   # k-projection chunks covering NQ
KCO = [0, 342, 684]
SCALE = D ** -0.5
# minimax cubic p(z) ~ exp(z) on [-0.55, 0.55], factored a3*(z-R)*(z^2+S*z+T);
# exp(y)*EB computed as (p(y/4) * a3*EB^(1/4))^4 in one custom DVE op.
EXP_R = -1.658048394110858
EXP_S = 1.462861309003841
EXP_T = 3.672443055287797
EXP_A3 = 0.1641584267735188
BF16 = ml_dtypes.bfloat16

_cache: dict = {}


def _register_exp4():
    """Register the fused quartic-exp custom DVE op: out = (p(z)*Src1)^4
    with p monic-factored; Src1 carries a3*EB^(1/4)."""
    if "exp4" in _cache:
        return _cache["exp4"]
    import numpy as np
    from concourse import dve_ops
    from concourse.dve_spec import Spec, Src0, Src1, C0, C1, C2
    from concourse.dve_table_gen import dve_ver_for
    from concourse.dve_uop import DveOpSpec

    name = "EXP4_EB_ANT"
    for op in dve_ops.OPS:
        if op.name == name:
            _cache["exp4"] = op
            return op

    m5 = (((Src0 + C1) * Src0) + C2) * (Src0 - C0) * Src1
    p2 = m5 * m5
    body = p2 * p2

    def ref(in0, in1, s0, s1, imm2):
        m5 = ((((in0 + s1) * in0) + imm2) * (in0 - s0) * in1).astype(np.float32)
        p2 = (m5 * m5).astype(np.float32)
        return (p2 * p2).astype(np.float32)

    op = dve_ops.DveOp(name, Spec(body=body, reference=ref), subdim=False,
                       uops_sha={})
    row = dve_ops._CUSTOM_DVE_ROW_BASE + len(dve_ops.OPS)
    assert row < 0x20
    dve_ops.OPS.append(op)
    dve_ops._SUB_OPCODE_FOR_NAME[name] = row
    dve_ops.CUSTOM_DVE_SPECS[name] = op.spec
    ver = dve_ver_for("TRN2")
    spec = DveOpSpec(name=name, opcode=row,
                     uops=dve_ops.lower(op.spec, ver=ver),
                     rd1_en=dve_ops.has_src1(op.spec))
    op.uops_sha[ver] = spec.sha(ver)
    _cache["exp4"] = op
    return op


def _build():
    import concourse.bass as bass
    import concourse.mybir as mybir
    import concourse.tile as tile
    from concourse import bacc

    dt = mybir.dt
    f32 = dt.float32
    bf = dt.bfloat16
    AFT = mybir.ActivationFunctionType

    exp4_op = _register_exp4()
    nc = bacc.Bacc("TRN2", target_bir_lowering=False, debug=False)

    xT_d = nc.declare_dram_parameter("xT", [BPC, C, NPAD], bf, isOutput=False)
    wqk_d = nc.declare_dram_parameter("wqk", [C, 2 * C], bf, isOutput=False)
    wv_d = nc.declare_dram_parameter("wv", [C, C], bf, isOutput=False)
    wp_d = nc.declare_dram_parameter("wp", [C, C], bf, isOutput=False)
    qkb_d = nc.declare_dram_parameter("qkb", [128, 2 * IB], f32, isOutput=False)
    vb_d = nc.declare_dram_parameter("vb", [128, C], f32, isOutput=False)
    pb_d = nc.declare_dram_parameter("pb", [128, C], f32, isOutput=False)
    eb_d = nc.declare_dram_parameter("eb", [H, NPAD, NQ], bf, isOutput=False)
    out_d = nc.declare_dram_parameter("out", [BPC, N, C], f32, isOutput=True)

    with tile.TileContext(nc) as tc:
        from contextlib import ExitStack

        ctx = ExitStack()
        with ctx:
            consts = ctx.enter_context(tc.tile_pool(name="consts", bufs=1))
            persist = ctx.enter_context(tc.tile_pool(name="persist", bufs=1))

            # ---- constants ----
            wqk_sb = consts.tile([128, IB, 2 * C], bf)
            nc.sync.dma_start(wqk_sb[:], wqk_d.ap().rearrange("(ib p) o -> p ib o", p=128))
            qkb_sb = consts.tile([128, 2 * IB], f32)
            nc.sync.dma_start(qkb_sb[:], qkb_d.ap())
            vb_sb = consts.tile([128, C], f32)
            nc.sync.dma_start(vb_sb[:], vb_d.ap())
            pb_sb = consts.tile([128, C], f32)
            nc.sync.dma_start(pb_sb[:], pb_d.ap())
            wv_sb = consts.tile([128, IB, C], bf)
            nc.sync.dma_start(wv_sb[:], wv_d.ap().rearrange("(ib p) o -> p ib o", p=128))
            wp_sb = consts.tile([128, IB, C], bf)
            nc.sync.dma_start(wp_sb[:], wp_d.ap().rearrange("(ib p) o -> p ib o", p=128))

            # ---- persistent tensors ----
            xT_sb = []
            v_sb = []   # v_sb[b][vc]: [128, KB, 6, D+1]
            k_sb = []   # k_sb[hp][b]: [128, NPAD]
            for b in range(BPC):
                t = persist.tile([128, IB, NPAD], bf, name=f"xT{b}")
                for ib in range(IB):
                    nc.sync.dma_start(
                        t[:, ib, :], xT_d.ap()[b][ib * 128:(ib + 1) * 128, :])
                xT_sb.append(t)
                vt = [persist.tile([128, KB, 6, D + 1], bf, name=f"v{b}_{vc}")
                      for vc in range(2)]
                for vc in range(2):
                    nc.gpsimd.memset(vt[vc][:, :, :, D:], 1.0)
                v_sb.append(vt)
            for hp in range(H // 2):
                k_sb.append([persist.tile([128, NPAD], bf, name=f"k{hp}_{b}")
                             for b in range(BPC)])
                for b in range(BPC):
                    nc.gpsimd.memset(k_sb[hp][b][:, NQ:], 0.0)

            # recip_dram[b*H + h, q] = bf16 reciprocal softmax denominators
            recip_dram = nc.dram_tensor("recip_dram", [BPC * H, NQ], bf)

            # a tiles per (b, qc): [128, 6, qcs]; y/proj resources
            a_pool = ctx.enter_context(tc.tile_pool(name="a", bufs=4))
            rb_pool = ctx.enter_context(tc.tile_pool(name="rb", bufs=1))
            y_pool = ctx.enter_context(tc.tile_pool(name="y", bufs=2))
            sums_pool = ctx.enter_context(tc.tile_pool(name="sums", bufs=2))
            stg_pool = ctx.enter_context(tc.tile_pool(name="stg", bufs=2))
            q_pool = ctx.enter_context(tc.tile_pool(name="q", bufs=3))
            eb_pool = ctx.enter_context(tc.tile_pool(name="ebp", bufs=3))
            ex_pool = ctx.enter_context(tc.tile_pool(name="exp", bufs=3))
            et_pool = ctx.enter_context(tc.tile_pool(name="etmp", bufs=4))
            mm_ps = ctx.enter_context(tc.tile_pool(name="mm_ps", bufs=2, space="PSUM"))
            s_ps = ctx.enter_context(tc.tile_pool(name="s_ps", bufs=4, space="PSUM"))
            o_ps = ctx.enter_context(tc.tile_pool(name="o_ps", bufs=2, space="PSUM"))

            a_sb = [[None] * 3 for _ in range(BPC)]

            def kproj(hp, b):
                """k rows for head pair hp, batch b -> k_sb[hp][b][:, :NQ]."""
                for kc in range(3):
                    ps = mm_ps.tile([128, 384], f32, name="mm")
                    for ib in range(IB):
                        nc.tensor.matmul(
                            ps[:, :KCS[kc]],
                            lhsT=wqk_sb[:, ib, C + hp * 128:C + (hp + 1) * 128],
                            rhs=xT_sb[b][:, ib, KCO[kc]:KCO[kc] + KCS[kc]],
                            start=(ib == 0),
                            stop=(ib == IB - 1),
                        )
                    nc.scalar.activation(
                        out=k_sb[hp][b][:, KCO[kc]:KCO[kc] + KCS[kc]],
                        in_=ps[:, :KCS[kc]],
                        func=AFT.Identity,
                        bias=qkb_sb[:, IB + hp:IB + hp + 1],
                        scale=1.0,
                    )

            def qproj(hp, qc, b, qt):
                """q chunk qc for head pair hp, batch b -> qt [128, qcs]."""
                qcs, qco = QCS[qc], QCO[qc]
                ps = mm_ps.tile([128, 384], f32, name="mm")
                for ib in range(IB):
                    nc.tensor.matmul(
                        ps[:, :qcs],
                        lhsT=wqk_sb[:, ib, hp * 128:(hp + 1) * 128],
                        rhs=xT_sb[b][:, ib, qco:qco + qcs],
                        start=(ib == 0),
                        stop=(ib == IB - 1),
                    )
                nc.scalar.activation(
                    out=qt[:, :qcs],
                    in_=ps[:, :qcs],
                    func=AFT.Identity,
                    bias=qkb_sb[:, hp:hp + 1],
                    scale=SCALE / 4,
                )

            def vproj(b, vc):
                """v for heads 6vc..6vc+5, batch b -> v_sb[b][vc]."""
                for kpb in range(KB):
                    ps = mm_ps.tile([128, 384], f32, name="mm")
                    for ib in range(IB):
                        nc.tensor.matmul(
                            ps[:],
                            lhsT=xT_sb[b][:, ib, kpb * 128:(kpb + 1) * 128],
                            rhs=wv_sb[:, ib, vc * 384:(vc + 1) * 384],
                            start=(ib == 0),
                            stop=(ib == IB - 1),
                        )
                    nc.vector.tensor_add(
                        out=v_sb[b][vc][:, kpb, :, :D],
                        in0=ps.rearrange("p (h d) -> p h d", d=D),
                        in1=vb_sb[:, vc * 384:(vc + 1) * 384].rearrange(
                            "p (h d) -> p h d", d=D),
                    )

            def attn_segment(hp, qc, b, qt, ebt, sums, mid=None):
                """scores+exp+PV for (hp, qc, b); writes a_sb and sums rows.
                `mid` emits extra tensor work between scores and PV so it
                overlaps the exp/mul phase."""
                qcs, qco = QCS[qc], QCO[qc]
                ex = [ex_pool.tile([128, KB, 384], bf, name="ex")
                      for _ in range(2)]
                et = None
                for kb in range(KB):
                    for par in range(2):
                        p0 = par * 64
                        st = s_ps.tile([128, 384], f32, name="st")
                        nc.tensor.matmul(
                            st[:, :qcs],
                            lhsT=k_sb[hp][b][p0:p0 + 64, kb * 128:(kb + 1) * 128],
                            rhs=qt[p0:p0 + 64, :qcs],
                        )
                        if par == 0:
                            nc.vector._custom_dve(
                                exp4_op,
                                out=ex[par][:, kb, :qcs],
                                in0=st[:, :qcs],
                                in1=ebt[par][:, kb, :qcs],
                                s0=EXP_R, s1=EXP_S, imm2=EXP_T,
                            )
                        else:
                            et = et_pool.tile([128, 384], bf, name="et")
                            nc.scalar.activation(
                                out=et[:, :qcs], in_=st[:, :qcs],
                                func=AFT.Exp, scale=4.0,
                            )
                            eng = nc.gpsimd if kb < 6 else nc.vector
                            eng.tensor_mul(
                                out=ex[par][:, kb, :qcs],
                                in0=et[:, :qcs],
                                in1=ebt[par][:, kb, :qcs],
                            )
                if mid is not None:
                    mid()
                for par in range(2):
                    h = 2 * hp + par
                    po = o_ps.tile([D + 1, 384], f32, name="po")
                    for kb in range(KB):
                        nc.tensor.matmul(
                            po[:, :qcs],
                            lhsT=v_sb[b][hp // 3][:, kb, (h - 6 * (hp // 3)), :],
                            rhs=ex[par][:, kb, :qcs],
                            start=(kb == 0),
                            stop=(kb == KB - 1),
                        )
                    stg = stg_pool.tile([65, 384], f32, name="stg")
                    nc.vector.tensor_copy(
                        out=stg[64:65, :qcs], in_=po[D:D + 1, :qcs])
                    nc.sync.dma_start(
                        sums[h:h + 1, :qcs], stg[64:65, :qcs])
                    nc.scalar.activation(
                        out=a_sb[b][qc][par * 64:(par + 1) * 64, hp, :qcs],
                        in_=po[:D, :qcs],
                        func=AFT.Copy,
                    )

            def finalize_sums(b, qc, sums):
                """reciprocal of denominators -> bf16 -> recip_dram rows."""
                qcs, qco = QCS[qc], QCO[qc]
                nc.vector.reciprocal_approx_fast(
                    out=sums[:, :qcs], in_=sums[:, :qcs])
                rqb = sums_pool.tile([H, 384], bf, name="rqb")
                nc.vector.tensor_copy(out=rqb[:, :qcs], in_=sums[:, :qcs])
                nc.sync.dma_start(
                    recip_dram.ap()[b * H:(b + 1) * H, qco:qco + qcs],
                    rqb[:, :qcs],
                )

            def norm_proj(b, qc):
                """normalize a_sb[b][qc] and project + emit output rows."""
                qcs, qco = QCS[qc], QCO[qc]
                rb = rb_pool.tile([128, 6, 384], bf, name="rb")
                base = recip_dram.ap()
                for par in range(2):
                    bcast = bass.AP(
                        tensor=base.tensor,
                        offset=(b * H + par) * NQ + qco,
                        ap=[[0, 64], [2 * NQ, IB], [1, qcs]],
                    )
                    nc.sync.dma_start(
                        rb[par * 64:(par + 1) * 64, :, :qcs], bcast)
                nc.vector.tensor_mul(
                    out=a_sb[b][qc][:, :, :qcs],
                    in0=a_sb[b][qc][:, :, :qcs],
                    in1=rb[:, :, :qcs],
                )
                for qt3 in range(3):
                    off = qt3 * 128
                    cols = min(128, qcs - off)
                    rows = min(cols, N - (qco + off))
                    if rows <= 0:
                        continue
                    yt = y_pool.tile([128, C], f32, name="yt")
                    for oc2 in range(2):
                        ps = mm_ps.tile([128, 384], f32, name="mm")
                        for ib in range(IB):
                            nc.tensor.matmul(
                                ps[:cols, :],
                                lhsT=a_sb[b][qc][:, ib, off:off + cols],
                                rhs=wp_sb[:, ib, oc2 * 384:(oc2 + 1) * 384],
                                start=(ib == 0),
                                stop=(ib == IB - 1),
                            )
                        nc.vector.tensor_add(
                            out=yt[:cols, oc2 * 384:(oc2 + 1) * 384],
                            in0=ps[:cols, :],
                            in1=pb_sb[:cols, oc2 * 384:(oc2 + 1) * 384],
                        )
                    nc.sync.dma_start(
                        out_d.ap()[b][qco + off:qco + off + rows, :],
                        yt[:rows, :],
                    )

            # ---- qc-major pipeline ----
            sums_t = [[None] * 3 for _ in range(BPC)]
            for qc in range(3):
                qcs, qco = QCS[qc], QCO[qc]
                for b in range(BPC):
                    sums_t[b][qc] = sums_pool.tile([H, 384], f32,
                                                   name="sums")
                    a_sb[b][qc] = a_pool.tile(
                        [128, 6, 384], bf, name="a")
                for hp in range(H // 2):
                    # eb tiles for this (hp, qc)
                    ebt = []
                    for par in range(2):
                        h = 2 * hp + par
                        t = eb_pool.tile([128, KB, 384], bf, name="ebt")
                        nc.sync.dma_start(
                            t[:, :, :qcs],
                            eb_d.ap()[h].rearrange(
                                "(kb p) q -> p kb q", p=128)[:, :, qco:qco + qcs],
                        )
                        ebt.append(t)
                    qts = []
                    for b in range(BPC):
                        if qc == 0:
                            kproj(hp, b)
                        qt = q_pool.tile([128, 384], bf, name="qt")
                        qproj(hp, qc, b, qt)
                        qts.append(qt)
                    for b in range(BPC):
                        # v projection rides between scores and PV so it
                        # overlaps the exp/mul phase of the same segment
                        mid = None
                        if qc == 0 and hp == 0:
                            mid = (lambda bb=b: vproj(bb, 0))
                        elif qc == 0 and hp == 1:
                            mid = (lambda bb=b: vproj(bb, 1))
                        attn_segment(hp, qc, b, qts[b], ebt,
                                     sums_t[b][qc], mid=mid)
                    if hp == 5:
                        for b in range(BPC):
                            finalize_sums(b, qc, sums_t[b][qc])
                    if qc > 0 and hp == 1:
                        norm_proj(0, qc - 1)
                    elif qc > 0 and hp == 3:
                        norm_proj(1, qc - 1)
            norm_proj(0, 2)
            norm_proj(1, 2)

    nc.compile()
    return nc


def _prepare_inputs(x, qkv_weight, q_bias, v_bias, rel_pos_table, proj_weight,
                    proj_bias, rel_pos_index):
    x = np.asarray(x, np.float32)
    qkv_weight = np.asarray(qkv_weight, np.float32)
    q_bias = np.asarray(q_bias, np.float32)
    v_bias = np.asarray(v_bias, np.float32)
    rel_pos_table = np.asarray(rel_pos_table, np.float32)
    proj_weight = np.asarray(proj_weight, np.float32)
    proj_bias = np.asarray(proj_bias, np.float32)
    rel_pos_index = np.asarray(rel_pos_index)

    wqk = np.ascontiguousarray(qkv_weight[:2 * C].T).astype(BF16)
    wv = np.ascontiguousarray(qkv_weight[2 * C:].T).astype(BF16)
    wp = np.ascontiguousarray(proj_weight.T).astype(BF16)

    qkb = np.concatenate([q_bias * np.float32(SCALE / 4), np.zeros(C, np.float32)])
    qkb = np.ascontiguousarray(qkb.reshape(2 * IB, 128).T)  # [128, 12]
    vb = np.ascontiguousarray(np.broadcast_to(v_bias, (128, C)))
    pb = np.ascontiguousarray(np.broadcast_to(proj_bias, (128, C)))

    # exp of transposed rel-pos bias, padded key rows = 0
    bias_qkh = rel_pos_table[rel_pos_index.reshape(-1)].reshape(N, N, H)
    bT = np.zeros((H, N, NQ), np.float64)
    bT[:, :, :N] = bias_qkh.transpose(2, 1, 0)
    ebt = np.zeros((H, NPAD, NQ), BF16)
    for h in range(H):
        if h % 2 == 0:
            ebt[h, :N, :] = (EXP_A3 * np.exp(bT[h] / 4)).astype(BF16)
        else:
            ebt[h, :N, :] = np.exp(bT[h]).astype(BF16)

    in_maps = []
    for core in range(NCORES):
        xb = x[core * BPC:(core + 1) * BPC]
        xT = np.zeros((BPC, C, NPAD), BF16)
        xT[:, :, :N] = xb.transpose(0, 2, 1).astype(BF16)
        in_maps.append({
            "xT": xT, "wqk": wqk, "wv": wv, "wp": wp,
            "qkb": qkb, "vb": vb, "pb": pb, "eb": ebt,
        })
    return in_maps


def kernel(**inputs) -> np.ndarray:
    from concourse.bass_utils import run_bass_kernel_spmd

    if "nc" not in _cache:
        _cache["nc"] = _build()
    nc = _cache["nc"]

    in_maps = _prepare_inputs(**inputs)
    trace = bool(_cache.get("trace", False))
    res = run_bass_kernel_spmd(nc, in_maps, core_ids=list(range(NCORES)),
                               trace=trace)
    _cache["last_results"] = res
    out = np.concatenate([r["out"] for r in res.results], axis=0)
    return out.astype(np.float32)
